# revision 1
# baseline (speedup 1.0000x reference)
"""
Trainium2 Bass kernel for a lower-triangular-masked GRU.

Math (per reference):
  lower = tril(ones(H,H)); WiG' = W_iG*lower + diag(b_iG); WhG' = W_hG*lower
  r = sigmoid(x @ Wir' + h @ Whr' + b_hr)
  z = sigmoid(x @ Wiz' + h @ Whz' + b_hz)
  n = tanh(x @ Win' + r * (h @ Whn' + b_hn))
  h' = h*z + (1-z)*n
  label = sigmoid(h' * W_out + b_out) * x ; ans[t,b] = max_h label >= 0.5 ? 1 : -1

Strategy: data-parallel over batch (B=64 -> 8 per core). Everything on
device runs in "hT layout": H on partitions (8 k-blocks of 128), batch on
the free dim, so the scan state needs no transposes. The x-projections
(pre = X @ Wi' + b_h{r,z}) are precomputed in a bulk phase (parallel over
time), stored to DRAM, and streamed into the sequential scan. The scan's
recurrent matmuls are W-stationary: out[j-block] = sum_k Wh'[k,j].T @ hT_k,
exploiting the triangular mask (only k >= j blocks are nonzero).
The label max over H is reduced on-device to per-partition maxes
(mbuf[p, t, b]); the host finishes the max over p and thresholds.
"""

import sys
import numpy as np
from contextlib import ExitStack

for _p in ("/opt/trn_rl_repo", "/root/.axon_site/_ro/trn_rl_repo"):
    if _p not in sys.path:
        sys.path.insert(0, _p)

import concourse.bass as bass
import concourse.tile as tile
from concourse import bacc
from concourse import mybir
from concourse.bass_utils import run_bass_kernel_spmd

T, B, H = 512, 64, 1024
NC = 8           # cores
BS = B // NC     # batch per core = 8
KB = H // 128    # 8 k-blocks
WIN = 32         # scan steps per For_i iteration
NW = T // WIN    # 16 windows

F32 = mybir.dt.float32
AF = mybir.ActivationFunctionType
ALU = mybir.AluOpType


def _build(b_out: float):
    nc = bacc.Bacc(None)

    xt_d = nc.declare_dram_parameter("xt", [KB, 128, T, BS], F32, isOutput=False)
    wih_d = nc.declare_dram_parameter("wih", [KB, 128, 3 * H], F32, isOutput=False)
    whh_d = nc.declare_dram_parameter("whh", [KB, 128, 3 * H], F32, isOutput=False)
    bpre_d = nc.declare_dram_parameter("bpre", [128, KB, 3], F32, isOutput=False)
    h0t_d = nc.declare_dram_parameter("h0t", [128, KB * BS], F32, isOutput=False)
    bhn_d = nc.declare_dram_parameter("bhn", [128, KB * BS], F32, isOutput=False)
    wout_d = nc.declare_dram_parameter("woutt", [128, KB * BS], F32, isOutput=False)
    eye_d = nc.declare_dram_parameter("eye", [128, 128], F32, isOutput=False)
    mbuf_d = nc.declare_dram_parameter("mbuf", [128, T, BS], F32, isOutput=True)

    with tile.TileContext(nc) as tc, ExitStack() as ctx:
        consts = ctx.enter_context(tc.tile_pool(name="consts", bufs=1))
        wpool = ctx.enter_context(tc.tile_pool(name="wpool", bufs=1))
        xtcp = ctx.enter_context(tc.tile_pool(name="xtcp", bufs=1))
        pop = ctx.enter_context(tc.tile_pool(name="pop", bufs=2))
        dramp = ctx.enter_context(tc.tile_pool(name="dramp", bufs=1, space="DRAM"))
        bpsum = ctx.enter_context(
            tc.tile_pool(name="bpsum", bufs=2, space=bass.MemorySpace.PSUM)
        )
        spsum = ctx.enter_context(
            tc.tile_pool(name="spsum", bufs=2, space=bass.MemorySpace.PSUM)
        )
        pwp = ctx.enter_context(tc.tile_pool(name="pwp", bufs=1))
        xwp = ctx.enter_context(tc.tile_pool(name="xwp", bufs=1))
        m1p = ctx.enter_context(tc.tile_pool(name="m1p", bufs=2))
        hp = ctx.enter_context(tc.tile_pool(name="hp", bufs=1))
        tp = ctx.enter_context(tc.tile_pool(name="tp", bufs=2))

        # pre[p, g, j, t, b] = (X @ Wi')[t, b, 128j+p] + b_h{r,z}[128j+p]
        pre_dram = dramp.tile([128, 3, KB, T, BS], F32)

        # weights tile: holds Wi' for bulk, then reloaded with Wh' for scan
        wsb = wpool.tile([128, KB, 3 * H], F32)
        nc.sync.dma_start(out=wsb[:], in_=wih_d[:].transpose([1, 0, 2]))

        bps = consts.tile([128, KB, 3], F32)
        nc.sync.dma_start(out=bps[:], in_=bpre_d[:])

        # ---------------- Phase A: bulk x-projections ----------------
        RC = 8  # row chunks of 512 rows (64 timesteps) each
        TC = T // RC  # 64 timesteps per chunk
        for rc in range(RC):
            xtc = xtcp.tile([128, KB, TC, BS], F32)
            nc.sync.dma_start(
                out=xtc[:],
                in_=xt_d[:, :, rc * TC : (rc + 1) * TC, :].transpose([1, 0, 2, 3]),
            )
            for g in range(3):
                for j in range(KB):
                    ps = bpsum.tile([128, TC, BS], F32)
                    for k in range(j, KB):
                        nc.tensor.matmul(
                            ps[:],
                            wsb[:, k, g * H + j * 128 : g * H + (j + 1) * 128],
                            xtc[:, k, :, :],
                            start=(k == j),
                            stop=(k == KB - 1),
                        )
                    po = pop.tile([128, TC, BS], F32)
                    if g == 2:
                        # pre_n has no bias
                        if (j % 2) == 0:
                            nc.scalar.copy(po[:], ps[:])
                        else:
                            nc.vector.tensor_copy(po[:], ps[:])
                    else:
                        if (j % 2) == 0:
                            nc.scalar.activation(
                                po[:], ps[:], AF.Identity, bias=bps[:, j, g : g + 1]
                            )
                        else:
                            nc.vector.tensor_scalar(
                                po[:], ps[:], bps[:, j, g : g + 1], None, ALU.add
                            )
                    nc.sync.dma_start(
                        out=pre_dram[:, g, j, rc * TC : (rc + 1) * TC, :], in_=po[:]
                    )

        # ---------------- Phase B: scan ----------------
        # reload weights tile with Wh' (Tile inserts WAR deps on wsb)
        nc.sync.dma_start(out=wsb[:], in_=whh_d[:].transpose([1, 0, 2]))

        eye = consts.tile([128, 128], F32)
        nc.sync.dma_start(out=eye[:], in_=eye_d[:])
        bconst = consts.tile([128, 2], F32)
        nc.vector.memset(bconst[:, 0:1], 1.0)
        nc.vector.memset(bconst[:, 1:2], b_out)
        bhn = consts.tile([128, KB * BS], F32)
        nc.sync.dma_start(out=bhn[:], in_=bhn_d[:])
        woutt = consts.tile([128, KB * BS], F32)
        nc.sync.dma_start(out=woutt[:], in_=wout_d[:])

        ht = hp.tile([128, KB * BS], F32)
        nc.sync.dma_start(out=ht[:], in_=h0t_d[:])

        with tc.For_i(0, NW, 1) as w:
            pw = pwp.tile([128, 3, KB, WIN, BS], F32)
            nc.sync.dma_start(out=pw[:], in_=pre_dram[:, :, :, bass.ts(w, WIN), :])
            xw = xwp.tile([128, KB, WIN, BS], F32)
            nc.sync.dma_start(
                out=xw[:], in_=xt_d[:, :, bass.ts(w, WIN), :].transpose([1, 0, 2, 3])
            )
            m1w = m1p.tile([128, WIN, BS], F32)

            for s in range(WIN):
                pr = spsum.tile([128, KB * BS], F32)
                pz = spsum.tile([128, KB * BS], F32)
                pn = spsum.tile([128, KB * BS], F32)
                # seed psums: pre_r, pre_z, b_hn (via identity matmul)
                nc.tensor.matmul(pr[:], eye[:], pw[:, 0, :, s, :], start=True, stop=False)
                nc.tensor.matmul(pz[:], eye[:], pw[:, 1, :, s, :], start=True, stop=False)
                nc.tensor.matmul(pn[:], eye[:], bhn[:], start=True, stop=False)
                # recurrent matmuls, W-stationary; r first, then n, then z
                for g, ps in ((0, pr), (2, pn), (1, pz)):
                    for j in range(KB):
                        for k in range(j, KB):
                            nc.tensor.matmul(
                                ps[:, j * BS : (j + 1) * BS],
                                wsb[:, k, g * H + j * 128 : g * H + (j + 1) * 128],
                                ht[:, k * BS : (k + 1) * BS],
                                start=False,
                                stop=(k == KB - 1),
                            )
                r = tp.tile([128, KB * BS], F32)
                nc.scalar.activation(r[:], pr[:], AF.Sigmoid)
                t1 = tp.tile([128, KB * BS], F32)
                nc.vector.tensor_mul(t1[:], r[:], pn[:])
                t2 = tp.tile([128, KB * BS], F32)
                nc.vector.tensor_add(
                    t2[:].rearrange("p (j b) -> p j b", j=KB),
                    t1[:].rearrange("p (j b) -> p j b", j=KB),
                    pw[:, 2, :, s, :],
                )
                n_ = tp.tile([128, KB * BS], F32)
                nc.scalar.activation(n_[:], t2[:], AF.Tanh)
                z = tp.tile([128, KB * BS], F32)
                nc.scalar.activation(z[:], pz[:], AF.Sigmoid)
                u = tp.tile([128, KB * BS], F32)
                nc.vector.tensor_mul(u[:], ht[:], z[:])
                w1 = tp.tile([128, KB * BS], F32)
                nc.scalar.activation(w1[:], z[:], AF.Identity, bias=bconst[:, 0:1], scale=-1.0)
                w2 = tp.tile([128, KB * BS], F32)
                nc.vector.tensor_mul(w2[:], w1[:], n_[:])
                nc.vector.tensor_add(ht[:], u[:], w2[:])
                # label path: sigmoid(h'*W_out + b_out) * x, then max over j
                v = tp.tile([128, KB * BS], F32)
                nc.vector.tensor_mul(v[:], ht[:], woutt[:])
                sv = tp.tile([128, KB * BS], F32)
                nc.scalar.activation(sv[:], v[:], AF.Sigmoid, bias=bconst[:, 1:2])
                lb = tp.tile([128, KB, BS], F32)
                nc.vector.tensor_mul(
                    lb[:], sv[:].rearrange("p (j b) -> p j b", j=KB), xw[:, :, s, :]
                )
                ma = tp.tile([128, 4, BS], F32)
                nc.vector.tensor_tensor(
                    ma[:], lb[:, 0:4, :], lb[:, 4:8, :], op=ALU.max
                )
                mb = tp.tile([128, 2, BS], F32)
                nc.vector.tensor_tensor(
                    mb[:], ma[:, 0:2, :], ma[:, 2:4, :], op=ALU.max
                )
                nc.vector.tensor_tensor(
                    m1w[:, s, :], mb[:, 0, :], mb[:, 1, :], op=ALU.max
                )
            nc.sync.dma_start(out=mbuf_d[:, bass.ts(w, WIN), :], in_=m1w[:])

    nc.compile()
    return nc


def kernel(
    input_, hidden0, W_ir, W_hr, W_iz, W_hz, W_in, W_hn,
    b_ir, b_hr, b_iz, b_hz, b_in, b_hn, W_out, b_out,
):
    input_ = np.ascontiguousarray(input_, dtype=np.float32)
    hidden0 = np.asarray(hidden0, dtype=np.float32)

    L = np.tril(np.ones((H, H), dtype=np.float32))
    wih = np.concatenate(
        [
            W_ir * L + np.diag(b_ir),
            W_iz * L + np.diag(b_iz),
            W_in * L + np.diag(b_in),
        ],
        axis=1,
    ).astype(np.float32).reshape(KB, 128, 3 * H)
    whh = np.concatenate([W_hr * L, W_hz * L, W_hn * L], axis=1).astype(
        np.float32
    ).reshape(KB, 128, 3 * H)
    bpre = np.stack(
        [
            b_hr.reshape(KB, 128).T,
            b_hz.reshape(KB, 128).T,
            np.zeros((128, KB), dtype=np.float32),
        ],
        axis=2,
    ).astype(np.float32)

    def rep_ht(vec):  # [H] -> [128, KB*BS] hT-layout replicated over batch
        return np.repeat(
            vec.reshape(KB, 128).T[:, :, None], BS, axis=2
        ).reshape(128, KB * BS).astype(np.float32)

    bhn_t = rep_ht(b_hn)
    wout_t = rep_ht(W_out)
    eye = np.eye(128, dtype=np.float32)

    nc = _build(float(np.asarray(b_out).reshape(-1)[0]))

    in_maps = []
    for c in range(NC):
        xc = input_[:, c * BS : (c + 1) * BS, :]  # [T, BS, H]
        xt = np.ascontiguousarray(xc.transpose(2, 0, 1)).reshape(KB, 128, T, BS)
        h0c = hidden0[c * BS : (c + 1) * BS, :]  # [BS, H]
        h0t = (
            np.ascontiguousarray(h0c.T)
            .reshape(KB, 128, BS)
            .transpose(1, 0, 2)
            .reshape(128, KB * BS)
        )
        in_maps.append(
            {
                "xt": xt,
                "wih": wih,
                "whh": whh,
                "bpre": bpre,
                "h0t": np.ascontiguousarray(h0t),
                "bhn": bhn_t,
                "woutt": wout_t,
                "eye": eye,
            }
        )

    res = run_bass_kernel_spmd(nc, in_maps, list(range(NC)))

    ans_f = np.empty((T, B), dtype=np.float32)
    for c in range(NC):
        mb = np.asarray(res.results[c]["mbuf"])  # [128, T, BS]
        ans_f[:, c * BS : (c + 1) * BS] = mb.max(axis=0)
    return np.where(ans_f >= 0.5, 1, -1).astype(np.int32)



# revision 13
# speedup vs baseline: 6.8840x; 6.8840x over previous
"""
Trainium2 Bass kernel for a lower-triangular-masked GRU.

Math (per reference):
  lower = tril(ones(H,H)); WiG' = W_iG*lower + diag(b_iG); WhG' = W_hG*lower
  r = sigmoid(x @ Wir' + h @ Whr' + b_hr)
  z = sigmoid(x @ Wiz' + h @ Whz' + b_hz)
  n = tanh(x @ Win' + r * (h @ Whn' + b_hn))
  h' = h*z + (1-z)*n
  label = sigmoid(h' * W_out + b_out) * x ; ans[t,b] = max_h label >= 0.5 ? 1 : -1

Strategy: data-parallel over batch (B=64 -> 8 per core). Everything on
device runs in "hT layout": H on partitions (8 k-blocks of 128), batch on
the free dim, so the scan state needs no transposes. The x-projections
(pre = X @ Wi' + b_h{r,z}) are precomputed in a bulk phase (parallel over
time) in fp16 matmuls (1 cycle/row vs 4 for fp32), stored to DRAM as fp16,
and streamed into the sequential scan. The scan's recurrent matmuls are
fp16 W-stationary: out[j-block] = sum_k Wh'[k,j].T @ hT_k, exploiting the
triangular mask (only k >= j blocks are nonzero). The hidden state is kept
in fp16 (values are bounded by the GRU's convex update, and fp16 keeps
0.05% rounding vs bf16's 0.4%).

Per step the elementwise work is split across engines to shorten the
critical chain  r -> t1 -> t2 -> n -> q2 -> h':
  Act:  r = sig(pr), z = sig(pz), oz = sig(-pz) (=1-z), n = tanh, sv
  DVE:  t1 = r*pn, t2 = t1+pre_n, q1 = z*h, q2 = oz*n, h' = q1+q2
  Pool: label path v = h'*W_out, lb = sv*x, per-window max tree over H
        blocks and a cross-partition (axis C) max, so the host only
        thresholds a [NW, WIN*BS] tensor per core.
"""

import sys
import numpy as np
from contextlib import ExitStack

for _p in ("/opt/trn_rl_repo", "/root/.axon_site/_ro/trn_rl_repo"):
    if _p not in sys.path:
        sys.path.insert(0, _p)

import concourse.bass as bass
import concourse.tile as tile
from concourse import bacc
from concourse import bass_isa
from concourse import mybir
from concourse.bass_utils import run_bass_kernel_spmd

T, B, H = 512, 64, 1024
NC = 8           # cores
BS = B // NC     # batch per core = 8
KB = H // 128    # 8 k-blocks
WIN = 32         # scan steps per For_i iteration
NW = T // WIN    # 16 windows

F32 = mybir.dt.float32
F16 = mybir.dt.float16
AF = mybir.ActivationFunctionType
ALU = mybir.AluOpType

# sim.py sets this to unroll the scan loop so TimelineSim (no_exec=True)
# can resolve control flow without an interpreter. Hardware runs use the
# For_i hardware loop (smaller instruction memory footprint).
SIM_UNROLL = False

LAST_RESULT = None


def _build(b_out: float):
    nc = bacc.Bacc(None)

    xt_d = nc.declare_dram_parameter("xt", [KB, 128, T, BS], F32, isOutput=False)
    wih_d = nc.declare_dram_parameter("wih", [KB, 128, 3 * H], F16, isOutput=False)
    whh_d = nc.declare_dram_parameter("whh", [KB, 128, 3 * H], F16, isOutput=False)
    bpre_d = nc.declare_dram_parameter("bpre", [128, KB, 3], F32, isOutput=False)
    h0t_d = nc.declare_dram_parameter("h0t", [128, KB * BS], F16, isOutput=False)
    bhn_d = nc.declare_dram_parameter("bhn", [128, KB * BS], F16, isOutput=False)
    wout_d = nc.declare_dram_parameter("woutt", [128, KB * BS], F32, isOutput=False)
    eye_d = nc.declare_dram_parameter("eye", [128, 128], F16, isOutput=False)
    mbuf_d = nc.declare_dram_parameter("mbuf", [128, T, BS], F32, isOutput=True)

    with tile.TileContext(nc) as tc, ExitStack() as ctx:
        consts = ctx.enter_context(tc.tile_pool(name="consts", bufs=1))
        wpool = ctx.enter_context(tc.tile_pool(name="wpool", bufs=1))
        xtcp = ctx.enter_context(tc.tile_pool(name="xtcp", bufs=1))
        xtcp16 = ctx.enter_context(tc.tile_pool(name="xtcp16", bufs=1))
        pop = ctx.enter_context(tc.tile_pool(name="pop", bufs=2))
        dramp = ctx.enter_context(tc.tile_pool(name="dramp", bufs=1, space="DRAM"))
        bpsum = ctx.enter_context(
            tc.tile_pool(name="bpsum", bufs=2, space=bass.MemorySpace.PSUM)
        )
        spsum = ctx.enter_context(
            tc.tile_pool(name="spsum", bufs=2, space=bass.MemorySpace.PSUM)
        )
        scr = ctx.enter_context(tc.tile_pool(name="scr", bufs=2))
        pwp = ctx.enter_context(tc.tile_pool(name="pwp", bufs=2))
        xwp = ctx.enter_context(tc.tile_pool(name="xwp", bufs=2))
        lbp = ctx.enter_context(tc.tile_pool(name="lbp", bufs=2))
        m1p = ctx.enter_context(tc.tile_pool(name="m1p", bufs=2))
        hp = ctx.enter_context(tc.tile_pool(name="hp", bufs=1))
        tp = ctx.enter_context(tc.tile_pool(name="tp", bufs=2))

        # pre[p, g, j, t, b] = (X @ Wi')[t, b, 128j+p] + b_h{r,z}[128j+p]
        pre_dram = dramp.tile([128, 3, KB, T, BS], F16)

        # weights tile: holds Wi' for bulk, then reloaded with Wh' for scan
        wsb = wpool.tile([128, KB, 3 * H], F16)
        nc.sync.dma_start(out=wsb[:], in_=wih_d[:].transpose([1, 0, 2]))

        bps = consts.tile([128, KB, 3], F32)
        nc.sync.dma_start(out=bps[:], in_=bpre_d[:])

        # ---------------- Phase A: bulk x-projections ----------------
        RC = 8  # row chunks of 512 rows (64 timesteps) each
        TC = T // RC  # 64 timesteps per chunk
        for rc in range(RC):
            xtc = xtcp.tile([128, KB, TC, BS], F32)
            nc.sync.dma_start(
                out=xtc[:],
                in_=xt_d[:, :, rc * TC : (rc + 1) * TC, :].transpose([1, 0, 2, 3]),
            )
            xtc16 = xtcp16.tile([128, KB, TC, BS], F16)
            nc.vector.tensor_copy(xtc16[:], xtc[:])
            for g in range(3):
                for j in range(KB):
                    ps = bpsum.tile([128, TC, BS], F32)
                    for k in range(j, KB):
                        nc.tensor.matmul(
                            ps[:],
                            wsb[:, k, g * H + j * 128 : g * H + (j + 1) * 128],
                            xtc16[:, k, :, :],
                            start=(k == j),
                            stop=(k == KB - 1),
                        )
                    po = pop.tile([128, TC, BS], F16)
                    if g == 2:
                        # pre_n has no bias
                        if (j % 2) == 0:
                            nc.scalar.copy(po[:], ps[:])
                        else:
                            nc.vector.tensor_copy(po[:], ps[:])
                    else:
                        if (j % 2) == 0:
                            nc.scalar.activation(
                                po[:], ps[:], AF.Identity, bias=bps[:, j, g : g + 1]
                            )
                        else:
                            nc.vector.tensor_scalar(
                                po[:], ps[:], bps[:, j, g : g + 1], None, ALU.add
                            )
                    nc.sync.dma_start(
                        out=pre_dram[:, g, j, rc * TC : (rc + 1) * TC, :], in_=po[:]
                    )

        # ---------------- Phase B: scan ----------------
        # reload weights tile with Wh' (Tile inserts WAR deps on wsb)
        nc.sync.dma_start(out=wsb[:], in_=whh_d[:].transpose([1, 0, 2]))

        eye = consts.tile([128, 128], F16)
        nc.sync.dma_start(out=eye[:], in_=eye_d[:])
        bconst = consts.tile([128, 1], F32)
        nc.vector.memset(bconst[:, 0:1], b_out)
        bhn = consts.tile([128, KB * BS], F16)
        nc.sync.dma_start(out=bhn[:], in_=bhn_d[:])
        woutt = consts.tile([128, KB * BS], F32)
        nc.sync.dma_start(out=woutt[:], in_=wout_d[:])

        ht = hp.tile([128, KB * BS], F16)
        nc.sync.dma_start(out=ht[:], in_=h0t_d[:])

        def emit_window(w):
            pw = pwp.tile([128, 3, KB, WIN, BS], F16)
            nc.sync.dma_start(out=pw[:], in_=pre_dram[:, :, :, bass.ts(w, WIN), :])
            xw = xwp.tile([128, KB, WIN, BS], F32)
            nc.sync.dma_start(
                out=xw[:], in_=xt_d[:, :, bass.ts(w, WIN), :].transpose([1, 0, 2, 3])
            )
            lbw = lbp.tile([128, WIN, KB, BS], F32)

            for s in range(WIN):
                pr = spsum.tile([128, KB * BS], F32)
                pz = spsum.tile([128, KB * BS], F32)
                pn = spsum.tile([128, KB * BS], F32)
                # seed psums: pre_r, pre_z, b_hn (via identity matmul, fp16)
                nc.tensor.matmul(
                    pr[:], eye[:], pw[:, 0, :, s, :], start=True, stop=False
                )
                nc.tensor.matmul(
                    pz[:], eye[:], pw[:, 1, :, s, :], start=True, stop=False
                )
                nc.tensor.matmul(pn[:], eye[:], bhn[:], start=True, stop=False)
                # recurrent matmuls, W-stationary; r first, then n, then z
                for g, ps in ((0, pr), (2, pn), (1, pz)):
                    for j in range(KB):
                        for k in range(j, KB):
                            nc.tensor.matmul(
                                ps[:, j * BS : (j + 1) * BS],
                                wsb[:, k, g * H + j * 128 : g * H + (j + 1) * 128],
                                ht[:, k * BS : (k + 1) * BS],
                                start=False,
                                stop=(k == KB - 1),
                            )
                # critical chain: r -> t1 -> t2 -> n -> q2 -> h'
                r = scr.tile([128, KB * BS], F32)
                nc.scalar.activation(r[:], pr[:], AF.Sigmoid)
                t1 = tp.tile([128, KB * BS], F32)
                nc.vector.tensor_mul(t1[:], r[:], pn[:])
                t2 = tp.tile([128, KB * BS], F32)
                nc.vector.tensor_add(
                    t2[:].rearrange("p (j b) -> p j b", j=KB),
                    t1[:].rearrange("p (j b) -> p j b", j=KB),
                    pw[:, 2, :, s, :],
                )
                n_ = tp.tile([128, KB * BS], F32)
                nc.scalar.activation(n_[:], t2[:], AF.Tanh)
                # off-chain: z and 1-z (= sigmoid(-pz)) from psum
                z = tp.tile([128, KB * BS], F32)
                nc.scalar.activation(z[:], pz[:], AF.Sigmoid)
                oz = scr.tile([128, KB * BS], F32)
                nc.scalar.activation(oz[:], pz[:], AF.Sigmoid, scale=-1.0)
                q1 = tp.tile([128, KB * BS], F32)
                nc.vector.tensor_mul(q1[:], z[:], ht[:])
                q2 = tp.tile([128, KB * BS], F32)
                nc.vector.tensor_mul(q2[:], oz[:], n_[:])
                nc.vector.tensor_add(ht[:], q1[:], q2[:])
                # label path on Pool: sigmoid(h'*W_out + b_out) * x
                v = tp.tile([128, KB * BS], F32)
                nc.gpsimd.tensor_mul(v[:], ht[:], woutt[:])
                sv = tp.tile([128, KB * BS], F32)
                nc.scalar.activation(sv[:], v[:], AF.Sigmoid, bias=bconst[:, 0:1])
                nc.gpsimd.tensor_mul(
                    lbw[:, s, :, :], sv[:].rearrange("p (j b) -> p j b", j=KB),
                    xw[:, :, s, :],
                )
            # per-window label reduction on DVE: max over KB blocks; the
            # max across partitions is finished on the host
            ma = m1p.tile([128, WIN, 4, BS], F32)
            nc.vector.tensor_tensor(
                ma[:], lbw[:, :, 0:4, :], lbw[:, :, 4:8, :], op=ALU.max
            )
            mb = m1p.tile([128, WIN, 2, BS], F32)
            nc.vector.tensor_tensor(
                mb[:], ma[:, :, 0:2, :], ma[:, :, 2:4, :], op=ALU.max
            )
            m1 = m1p.tile([128, WIN, BS], F32)
            nc.vector.tensor_tensor(
                m1[:], mb[:, :, 0, :], mb[:, :, 1, :], op=ALU.max
            )
            nc.sync.dma_start(out=mbuf_d[:, bass.ts(w, WIN), :], in_=m1[:])

        if SIM_UNROLL:
            for w in range(NW):
                emit_window(w)
        else:
            with tc.For_i(0, NW, 1) as w:
                emit_window(w)

    nc.compile()
    return nc


def kernel(
    input_, hidden0, W_ir, W_hr, W_iz, W_hz, W_in, W_hn,
    b_ir, b_hr, b_iz, b_hz, b_in, b_hn, W_out, b_out,
):
    input_ = np.ascontiguousarray(input_, dtype=np.float32)
    hidden0 = np.asarray(hidden0, dtype=np.float32)

    L = np.tril(np.ones((H, H), dtype=np.float32))
    wih = np.concatenate(
        [
            W_ir * L + np.diag(b_ir),
            W_iz * L + np.diag(b_iz),
            W_in * L + np.diag(b_in),
        ],
        axis=1,
    ).astype(np.float16).reshape(KB, 128, 3 * H)
    whh = np.concatenate([W_hr * L, W_hz * L, W_hn * L], axis=1).astype(
        np.float16
    ).reshape(KB, 128, 3 * H)
    bpre = np.stack(
        [
            b_hr.reshape(KB, 128).T,
            b_hz.reshape(KB, 128).T,
            np.zeros((128, KB), dtype=np.float32),
        ],
        axis=2,
    ).astype(np.float32)

    def rep_ht(vec, dt):  # [H] -> [128, KB*BS] hT-layout replicated over batch
        return np.repeat(
            vec.reshape(KB, 128).T[:, :, None], BS, axis=2
        ).reshape(128, KB * BS).astype(dt)

    bhn_t = rep_ht(b_hn, np.float16)
    wout_t = rep_ht(W_out, np.float32)
    eye = np.eye(128, dtype=np.float16)

    nc = _build(float(np.asarray(b_out).reshape(-1)[0]))

    in_maps = []
    for c in range(NC):
        xc = input_[:, c * BS : (c + 1) * BS, :]  # [T, BS, H]
        xt = np.ascontiguousarray(xc.transpose(2, 0, 1)).reshape(KB, 128, T, BS)
        h0c = hidden0[c * BS : (c + 1) * BS, :]  # [BS, H]
        h0t = (
            np.ascontiguousarray(h0c.T)
            .reshape(KB, 128, BS)
            .transpose(1, 0, 2)
            .reshape(128, KB * BS)
        )
        in_maps.append(
            {
                "xt": xt,
                "wih": wih,
                "whh": whh,
                "bpre": bpre,
                "h0t": np.ascontiguousarray(h0t).astype(np.float16),
                "bhn": bhn_t,
                "woutt": wout_t,
                "eye": eye,
            }
        )

    res = run_bass_kernel_spmd(nc, in_maps, list(range(NC)))
    global LAST_RESULT
    LAST_RESULT = res

    ans_f = np.empty((T, B), dtype=np.float32)
    for c in range(NC):
        mb = np.asarray(res.results[c]["mbuf"])  # [128, T, BS]
        ans_f[:, c * BS : (c + 1) * BS] = mb.max(axis=0)
    return np.where(ans_f >= 0.5, 1, -1).astype(np.int32)
